# revision 1
# baseline (speedup 1.0000x reference)
"""Trainium2 Bass kernel for nn_Block_16544214024520 (dense_cnn).

Data-parallel over batch: 16 samples -> 2 per NeuronCore x 8 cores.
All parameters replicated. Per-sample layout: channels on partitions
(256 = 2 chunks of 128), pixels (64x64 = 4096) on the free dim.

Reference pipeline (per sample):
  gn(32) -> 1x1 conv(256->256)+silu -> gn(16) -> 3x3 grouped conv
  (g=4, 256->512)+silu -> gn(2) -> window-mean(8x8) -> radix amax ->
  1x1 g-conv(256->64)+silu -> gn(8) -> 1x1 g-conv(64->512) ->
  softmax over radix(2) -> gated combine -> channel matmul(256->256)
  -> gn(32) -> +residual
"""

import os
import sys

for _p in ("/opt/trn_rl_repo", "/opt/pypackages"):
    if _p not in sys.path:
        sys.path.append(_p)

import ml_dtypes
import numpy as np

import concourse.bass as bass  # noqa: F401
import concourse.mybir as mybir
import concourse.tile as tile
from concourse import bacc
from concourse.masks import make_identity

F32 = mybir.dt.float32
F32R = mybir.dt.float32r
BF16 = mybir.dt.bfloat16
AF = mybir.ActivationFunctionType
ALU = mybir.AluOpType
AX = mybir.AxisListType

NCORES = 8
BPC = 2          # samples per core
C = 256          # channels
H = W = 64
NPIX = H * W     # 4096
PADW = W + 2     # 66
Hn = Wn = 8      # window grid
WS = 8           # window size
EPS = 1e-5
NT = 8           # n-tiles of 512 pixels (8 rows of 64)


# ---------------------------------------------------------------- host prep

def _host_consts():
    """Constant matrices shared by all cores (built once)."""
    c = {}
    # GN over 256 channels, 32 groups of 8 (GN1/GN2/GN5)
    gm1 = np.zeros((2, 128, 32), np.float32)
    rep1 = np.zeros((2, 128, 128), np.float32)
    for ch in range(2):
        for k in range(128):
            g = (128 * ch + k) // 8
            gm1[ch, k, g] = 1.0 / 8.0
        for m in range(128):
            rep1[ch, (128 * ch + m) // 8 % 128, m] = 1.0
    c["gm1"] = gm1
    c["rep1"] = rep1
    # GN2: 16 groups of 16 over 256 channels
    gm2 = np.zeros((2, 128, 16), np.float32)
    rep2 = np.zeros((2, 128, 128), np.float32)
    for ch in range(2):
        for k in range(128):
            gm2[ch, k, (128 * ch + k) // 16] = 1.0 / 16.0
        for m in range(128):
            rep2[ch, (128 * ch + m) // 16, m] = 1.0
    c["gm2"] = gm2
    c["rep2"] = rep2
    # GN3 over 512 channels, 2 groups of 256 (chunks 0,1 -> g0; 2,3 -> g1)
    g3 = np.zeros((4, 128, 2), np.float32)
    r3 = np.zeros((4, 128, 128), np.float32)
    for mc in range(4):
        g3[mc, :, mc // 2] = 1.0 / 256.0
        r3[mc, mc // 2, :] = 1.0
    c["g3"] = g3
    c["r3"] = r3
    # GN4 over 64 channels, 8 groups of 8
    g4 = np.zeros((128, 8), np.float32)
    for k in range(64):
        g4[k, k // 8] = 1.0 / 8.0
    r4 = np.zeros((128, 64), np.float32)
    for m in range(64):
        r4[m // 8, m] = 1.0
    c["g4"] = g4
    c["r4"] = r4
    return c


def _host_weights(w0, b0, w1, b1, w2, b2, w3, b3, weight):
    """Rearrange torch-layout conv weights into matmul lhsT tensors."""
    d = {}
    # conv0: out[o,p] = sum_i w0[o,i] x[i,p]  -> lhsT[i,o]
    d["w0T"] = np.ascontiguousarray(w0[:, :, 0, 0].T).astype(
        ml_dtypes.bfloat16)  # [256,256]
    d["b0c"] = np.ascontiguousarray(b0.reshape(C, 1)).astype(np.float32)
    # conv1: grouped 3x3, groups=4 (in 64 -> out 128 each), natural order.
    # w1t[tap, kc] is a [128, 256] block: rows = in-ch of groups (2kc, 2kc+1),
    # col block 0 (0:128) = out chunk 2kc (uses rows 0:64),
    # col block 1 (128:256) = out chunk 2kc+1 (uses rows 64:128).
    w1t = np.zeros((9, 2, 128, 256), np.float32)
    for tap in range(9):
        dy, dx = tap // 3, tap % 3
        for kc in range(2):
            for blk in range(2):
                g = 2 * kc + blk
                # out channels g*128 + j, in-ch local r in 0..63
                w1t[tap, kc, blk * 64:(blk + 1) * 64, blk * 128:(blk + 1) * 128] = \
                    w1[g * 128:(g + 1) * 128, :, dy, dx].T
    d["w1t"] = w1t.astype(ml_dtypes.bfloat16)
    d["b1c"] = np.ascontiguousarray(b1.reshape(2 * C, 1)).astype(np.float32)
    # conv2: groups=2 (in 128 -> out 32); fold the 1/64 window mean here.
    w2t = np.zeros((2, 128, 32), np.float32)
    for g in range(2):
        w2t[g] = w2[g * 32:(g + 1) * 32, :, 0, 0].T
    d["w2t"] = w2t
    d["b2c"] = np.ascontiguousarray(b2.reshape(64, 1)).astype(np.float32)
    # conv3: groups=2 (in 32 -> out 256); K padded to 128 with zero rows.
    w3t = np.zeros((4, 128, 128), np.float32)
    for g in range(4):
        src = w3[g * 128:(g + 1) * 128, :, 0, 0]      # [128, 32]
        r0 = 0 if g < 2 else 32
        w3t[g, r0:r0 + 32, :] = src.T
    d["w3t"] = w3t
    # final einsum: out[c,p] = sum_C weight[C,c] z[C,p], z[C] = zint[2C]+zint[2C+1]
    # fold the radix pair-sum by duplicating rows: wdup[c512, c] = weight[c512//2, c]
    wdup = np.repeat(weight.astype(np.float32), 2, axis=0)   # [512, 256]
    d["wdupT"] = np.ascontiguousarray(wdup).astype(ml_dtypes.bfloat16)
    return d


def _pack_consts(wd, cm):
    """Pack all fp32 constants into one [128, F] tensor and all bf16
    weights into another, so startup needs only two DMAs."""
    fcols = []   # list of [128, n] fp32 blocks
    def addf(x):
        x = np.asarray(x, np.float32)
        assert x.shape[0] == 128
        fcols.append(x.reshape(128, -1))
    for c in range(2):
        addf(cm["gm1"][c]); addf(cm["rep1"][c])
        addf(cm["gm2"][c]); addf(cm["rep2"][c])
    for g in range(4):
        addf(cm["g3"][g]); addf(cm["r3"][g])
    addf(cm["g4"]); addf(cm["r4"])
    b0 = wd["b0c"].reshape(2, 128, 1)
    addf(b0[0]); addf(b0[1])
    b1 = wd["b1c"].reshape(4, 128, 1)
    for g in range(4):
        addf(b1[g])
    b2p = np.zeros((128, 1), np.float32)
    b2p[0:64] = wd["b2c"]
    addf(b2p)
    addf(np.full((128, 1), EPS, np.float32))
    for g in range(2):
        addf(wd["w2t"][g])
    for g in range(4):
        addf(wd["w3t"][g])
    cpack = np.concatenate(fcols, axis=1)
    bcols = [np.asarray(wd["w0T"], ml_dtypes.bfloat16).reshape(128, -1,
                                                               order="F")
             ]
    # w0T is [256, 256] -> two chunks [128, 256]
    w0 = np.asarray(wd["w0T"])
    bcols = [w0[0:128], w0[128:256]]
    w1 = np.asarray(wd["w1t"])   # [9, 2, 128, 256]
    for t in range(9):
        for k in range(2):
            bcols.append(w1[t, k])
    wdp = np.asarray(wd["wdupT"])
    for k in range(4):
        bcols.append(wdp[k * 128:(k + 1) * 128])
    bpack = np.concatenate(bcols, axis=1).astype(ml_dtypes.bfloat16)
    return cpack, bpack


# ---------------------------------------------------------------- builder

def build_nc(sim_safe: bool = False):
    nc = bacc.Bacc("TRN2", target_bir_lowering=False, debug=False,
                   num_devices=NCORES)

    def din(name, shape, dt=F32):
        return nc.dram_tensor(name, list(shape), dt, kind="ExternalInput").ap()

    hs = din("hs", (BPC, C, H, W))
    hsb = din("hsb", (BPC, C, H, W), BF16)
    NCF = 32 + 128 + 16 + 128 + 32 + 128 + 16 + 128 + 4 * (2 + 128) \
        + 8 + 64 + 2 + 4 + 1 + 1 + 2 * 32 + 4 * 128
    NBF = 256 * 2 + 9 * 2 * 256 + 4 * 256
    cpack_d = din("cpack", (128, NCF))
    bpack_d = din("bpack", (128, NBF), BF16)

    out_d = nc.dram_tensor("out", [BPC, C, H, W], F32, kind="ExternalOutput").ap()

    with tile.TileContext(nc) as tc:
        with tc.tile_pool(name="consts", bufs=1) as cst, \
             tc.tile_pool(name="big", bufs=1) as big, \
             tc.tile_pool(name="small", bufs=2) as sm, \
             tc.tile_pool(name="psum", bufs=2, space="PSUM") as psp:

            # ---- load constants / weights (two packed DMAs) ----
            cpk = cst.tile([128, NCF], F32, name="cpk")
            nc.sync.dma_start(out=cpk, in_=cpack_d)
            bpk = cst.tile([128, NBF], BF16, name="bpk")
            nc.sync.dma_start(out=bpk, in_=bpack_d)

            class _Cur:
                def __init__(self):
                    self.o = 0
            _cf, _cb = _Cur(), _Cur()

            def fsl(n):
                s = cpk[:, _cf.o:_cf.o + n]
                _cf.o += n
                return s

            def bsl(n):
                s = bpk[:, _cb.o:_cb.o + n]
                _cb.o += n
                return s

            gm1_t, rep1_t, gm2_t, rep2_t = [], [], [], []
            for c in range(2):
                gm1_t.append(fsl(32)); rep1_t.append(fsl(128))
                gm2_t.append(fsl(16)); rep2_t.append(fsl(128))
            g3_t, r3_t = [], []
            for g in range(4):
                g3_t.append(fsl(2)); r3_t.append(fsl(128))
            g4_t = fsl(8); r4_t = fsl(64)
            b0_t = [fsl(1) for _ in range(2)]
            b1_t = [fsl(1) for _ in range(4)]
            b2_t = fsl(1)
            eps_t = fsl(1)
            w2_t = [fsl(32) for _ in range(2)]
            w3_t = [fsl(128) for _ in range(4)]
            assert _cf.o == NCF
            w0_t = [bsl(256) for _ in range(2)]
            w1_t = [[None, None] for _ in range(9)]
            for t in range(9):
                for k in range(2):
                    w1_t[t][k] = bsl(256)
            wd_t = [bsl(256) for _ in range(4)]
            assert _cb.o == NBF
            ident = cst.tile([128, 128], F32, name="ident")
            make_identity(nc, ident)

            # ------------------------------------------------ helpers
            def silu_evac(out_ap, psum_ap, bias_ap, tag):
                """out = silu(psum + bias); fused on HW, 2-op in CoreSim."""
                if not sim_safe:
                    nc.scalar.activation(out=out_ap, in_=psum_ap, func=AF.Silu,
                                         bias=bias_ap, scale=1.0)
                else:
                    sgf = sm.tile([128, 512], F32, tag="sg", bufs=2,
                                  name=f"sg_{tag}", uniquify=True)
                    pp = psum_ap.partition_size()
                    ff = psum_ap.free_size()
                    sgt = sgf[0:pp, 0:ff]
                    nc.scalar.activation(out=sgt, in_=psum_ap, func=AF.Sigmoid,
                                         bias=bias_ap, scale=1.0)
                    nc.vector.scalar_tensor_tensor(
                        out=out_ap, in0=psum_ap, scalar=bias_ap, in1=sgt,
                        op0=ALU.add, op1=ALU.mult)

            def gn_scale_bias(mvs, gmat_list, rmat_list, ngroups, tag,
                              ncols=2):
                """Per-channel (scale, bias) tiles for a group norm.

                mvs: list of [128, 2] SBUF tiles of per-channel (mean, var),
                valid on the partition ranges covered by gmat rows.
                Returns list of [128, 2] tiles (col0 = rstd, col1 = -mean*rstd)
                replicated back to channels, one per input chunk.
                """
                nchunk = len(mvs)
                # per-channel [mean, E[x^2]]
                rstats = []
                for ci, mv in enumerate(mvs):
                    r = sm.tile([128, 2], F32, tag=f"r_{tag}", bufs=2 * nchunk)
                    nc.vector.tensor_copy(out=r[:, 0:1], in_=mv[:, 0:1])
                    nc.vector.scalar_tensor_tensor(
                        out=r[:, 1:2], in0=mv[:, 0:1], scalar=mv[:, 0:1],
                        in1=mv[:, 1:2], op0=ALU.mult, op1=ALU.add)
                    rstats.append(r)
                pg = psp.tile([128, 2], F32, tag="gn_ps", bufs=1)
                for ci in range(nchunk):
                    nc.tensor.matmul(pg[0:ngroups, :], gmat_list[ci], rstats[ci],
                                     start=(ci == 0), stop=(ci == nchunk - 1))
                gt = sm.tile([128, 2], F32, tag=f"gt_{tag}", bufs=2)
                nc.vector.memset(gt, 0.0)
                nc.scalar.copy(out=gt[0:ngroups, :], in_=pg[0:ngroups, :])
                # -var = mean^2 - E[x^2]
                negv = sm.tile([128, 1], F32, tag=f"nv_{tag}", bufs=2)
                nc.vector.scalar_tensor_tensor(
                    out=negv[0:ngroups], in0=gt[0:ngroups, 0:1],
                    scalar=gt[0:ngroups, 0:1], in1=gt[0:ngroups, 1:2],
                    op0=ALU.mult, op1=ALU.subtract)
                sd = sm.tile([128, 1], F32, tag=f"sd_{tag}", bufs=2)
                nc.scalar.activation(out=sd[0:ngroups], in_=negv[0:ngroups],
                                     func=AF.Sqrt, bias=eps_t[0:ngroups],
                                     scale=-1.0)
                rstd = sm.tile([128, 1], F32, tag=f"rs_{tag}", bufs=2)
                nc.vector.reciprocal(out=rstd[0:ngroups], in_=sd[0:ngroups])
                stg = sm.tile([128, 3], F32, tag=f"st_{tag}", bufs=2)
                nc.vector.memset(stg, 0.0)
                nc.vector.tensor_copy(out=stg[0:ngroups, 0:1], in_=rstd[0:ngroups])
                nc.vector.tensor_scalar(
                    out=stg[0:ngroups, 1:2], in0=gt[0:ngroups, 0:1],
                    scalar1=rstd[0:ngroups], scalar2=-1.0,
                    op0=ALU.mult, op1=ALU.mult)
                if ncols == 3:
                    # col2 = -mean (for deferred-scale group norm)
                    nc.vector.tensor_scalar(
                        out=stg[0:ngroups, 2:3], in0=gt[0:ngroups, 0:1],
                        scalar1=-1.0, scalar2=None, op0=ALU.mult)
                scs = []
                for ci, rmat in enumerate(rmat_list):
                    mm = rmat.shape[-1]
                    pr = psp.tile([128, 3], F32, tag="gn_ps", bufs=1)
                    nc.tensor.matmul(pr[0:mm, 0:ncols], rmat,
                                     stg[:, 0:ncols], start=True, stop=True)
                    sc = sm.tile([128, 3], F32, tag=f"sc_{tag}",
                                 bufs=2 * nchunk)
                    nc.scalar.copy(out=sc[0:mm, 0:ncols], in_=pr[0:mm, 0:ncols])
                    scs.append(sc)
                return scs

            def chan_stats(src_list, tag, nsub=NT):
                """bn_stats/bn_aggr per chunk -> [128,2] (mean, var) tiles."""
                mvs = []
                for ci, src in enumerate(src_list):
                    if src.dtype == F32R:
                        src = src.bitcast(F32)
                    free = src.free_size()
                    sub = free // 512 if free >= 512 else 1
                    bst = sm.tile([128, max(sub, 1), 6], F32,
                                  tag=f"bst_{tag}", bufs=2)
                    if sub > 1:
                        srcv = src.rearrange("p (a b) -> p a b", a=sub)
                        for si in range(sub):
                            nc.vector.bn_stats(out=bst[:, si, :],
                                               in_=srcv[:, si, :])
                    else:
                        nc.vector.bn_stats(out=bst,
                                           in_=src.unsqueeze(1))
                    mv = sm.tile([128, 2], F32, tag=f"mv_{tag}",
                                 bufs=2 * len(src_list))
                    nc.vector.bn_aggr(out=mv, in_=bst)
                    mvs.append(mv)
                return mvs

            # ------------------------------------------------ per-sample body
            for b in range(BPC):
                hsv = hs[b].rearrange("c h w -> c (h w)")   # [256, 4096]

                _s, _ = nc.enter_named_scope(f"ld_gn1_{b}", False)
                # ---- load input (sliced DMAs) + GN1 stats in-loop ----
                xw = [big.tile([128, NPIX], BF16, tag="xw", bufs=4,
                               padded_shape=[128, PADW * PADW],
                               name=f"xw{b}_{i}") for i in range(2)]
                hsbv = hsb[b].rearrange("c h w -> c (h w)")
                bst1 = [sm.tile([128, NT, 6], F32, tag="bst1", bufs=2,
                                name=f"bst1_{b}_{i}") for i in range(2)]
                for c in range(2):
                    nc.sync.dma_start(out=xw[c],
                                      in_=hsbv[c * 128:(c + 1) * 128, :])
                    for n in range(NT):
                        nsl = bass.ts(n, 512)
                        nc.vector.bn_stats(out=bst1[c][:, n, :],
                                           in_=xw[c][:, nsl])
                mv1 = []
                for c in range(2):
                    mv = sm.tile([128, 2], F32, tag="mv1", bufs=2,
                                 name=f"mv1_{b}_{c}")
                    nc.vector.bn_aggr(out=mv, in_=bst1[c])
                    mv1.append(mv)
                sc1 = gn_scale_bias(mv1, gm1_t, rep1_t, 32, "gn1")
                # preload the fp32 residual input early (DMA is idle here)
                xr = [big.tile([128, NPIX], F32, tag="xr", bufs=2,
                               name=f"xr{b}_{i}") for i in range(2)]
                for c in range(2):
                    nc.sync.dma_start(out=xr[c],
                                      in_=hsv[c * 128:(c + 1) * 128, :])

                # fold GN1 into conv0 weights
                w0s = [sm.tile([128, 256], BF16, tag="w0s", bufs=2,
                               name=f"w0s{b}_{i}") for i in range(2)]
                t1b = [sm.tile([128, 1], BF16, tag="t1b", bufs=2,
                               name=f"t1b{b}_{i}") for i in range(2)]
                for c in range(2):
                    nc.vector.tensor_scalar_mul(out=w0s[c], in0=w0_t[c],
                                                scalar1=sc1[c][:, 0:1])
                b0p = [sm.tile([128, 1], F32, tag="b0p", bufs=2,
                               name=f"b0p{b}_{i}") for i in range(2)]
                for c in range(2):
                    nc.vector.tensor_copy(out=t1b[c], in_=sc1[c][:, 1:2])
                for m in range(2):
                    pb = psp.tile([128, 1], F32, tag="gn_ps", bufs=1)
                    for kc in range(2):
                        nc.tensor.matmul(
                            pb,
                            w0s[kc][:, m * 128:(m + 1) * 128],
                            t1b[kc],
                            start=(kc == 0), stop=(kc == 1))
                    nc.scalar.activation(out=pb if False else b0p[m], in_=pb,
                                         func=AF.Identity, bias=b0_t[m],
                                         scale=1.0)

                nc.leave_named_scope(f"ld_gn1_{b}", _s, False)
                _s, _ = nc.enter_named_scope(f"conv0_{b}", False)
                # ---- conv0 (1x1) + silu -> y0; GN2 stats in-loop ----
                y0 = [big.tile([128, NPIX], BF16, tag="y0", bufs=3,
                               name=f"y0{b}_{i}") for i in range(2)]
                bst2 = [sm.tile([128, NT, 6], F32, tag="bst2", bufs=2,
                                name=f"bst2_{b}_{i}") for i in range(2)]
                for m in range(2):
                    for ng in range(2):
                        pts0 = [psp.tile([128, 512], F32, tag="acc", bufs=6,
                                         name=f"pc0_{b}_{m}_{ng}_{i}")
                                for i in range(4)]
                        for ni in range(4):
                            n = ng * 4 + ni
                            for kc in range(2):
                                nc.tensor.matmul(
                                    pts0[ni],
                                    w0s[kc][:, m * 128:(m + 1) * 128],
                                    xw[kc][:, bass.ts(n, 512)],
                                    start=(kc == 0), stop=(kc == 1))
                        for ni in range(4):
                            n = ng * 4 + ni
                            nsl = bass.ts(n, 512)
                            silu_evac(y0[m][:, nsl], pts0[ni], b0p[m], "c0")
                            nc.vector.bn_stats(out=bst2[m][:, n, :],
                                               in_=y0[m][:, nsl])
                mv2 = []
                for c in range(2):
                    mv = sm.tile([128, 2], F32, tag="mv2", bufs=2,
                                 name=f"mv2_{b}_{c}")
                    nc.vector.bn_aggr(out=mv, in_=bst2[c])
                    mv2.append(mv)
                sc2 = gn_scale_bias(mv2, gm2_t, rep2_t, 16, "gn2")
                # GN2 apply into the padded conv1 input buffer
                xp = [big.tile([128, PADW, PADW], BF16, tag="xw", bufs=4,
                               name=f"xp{b}_{i}") for i in range(2)]
                for c in range(2):
                    xpf = xp[c]
                    nc.gpsimd.memset(xpf[:, 0:1, :], 0.0)
                    nc.gpsimd.memset(xpf[:, PADW - 1:PADW, :], 0.0)
                    nc.gpsimd.memset(xpf[:, 1:PADW - 1, 0:1], 0.0)
                    nc.gpsimd.memset(xpf[:, 1:PADW - 1, PADW - 1:PADW], 0.0)
                    nc.gpsimd.tensor_scalar(
                        out=xp[c][:, 1:H + 1, 1:W + 1],
                        in0=y0[c].rearrange("p (h w) -> p h w", h=H),
                        scalar1=sc2[c][:, 0:1], scalar2=sc2[c][:, 1:2],
                        op0=ALU.mult, op1=ALU.add)

                nc.leave_named_scope(f"conv0_{b}", _s, False)
                _s, _ = nc.enter_named_scope(f"conv1_{b}", False)
                # ---- conv1 (3x3 grouped) + silu -> y1; stats + pool in-loop ----
                y1 = [big.tile([128, NPIX], BF16, tag="y1", bufs=4,
                               name=f"y1{b}_{i}") for i in range(4)]
                bst3 = [sm.tile([128, NT, 6], F32, tag="bst3", bufs=4,
                                name=f"bst3_{b}_{i}") for i in range(4)]
                pa = [sm.tile([128, NT * 64], BF16, tag="pa", bufs=4,
                              name=f"pa{b}_{i}") for i in range(4)]
                for np_ in range(4):
                    for kc in range(2):
                        pts = [[psp.tile([128, 512], F32, tag="acc", bufs=6,
                                         name=f"pc1_{b}_{np_}_{kc}_{ni}_{blk}")
                                for blk in range(2)] for ni in range(2)]
                        for ni in range(2):
                            n = np_ * 2 + ni
                            r0 = n * WS
                            for tap in range(9):
                                dy, dx = tap // 3 - 1, tap % 3 - 1
                                for blk in range(2):
                                    p0 = blk * 64
                                    rhs = xp[kc][p0:p0 + 64,
                                                 r0 + 1 + dy:r0 + 9 + dy,
                                                 1 + dx:W + 1 + dx]
                                    lhsT = w1_t[tap][kc][
                                        p0:p0 + 64, blk * 128:(blk + 1) * 128]
                                    nc.tensor.matmul(
                                        pts[ni][blk], lhsT, rhs,
                                        start=(tap == 0), stop=(tap == 8))
                        for ni in range(2):
                            n = np_ * 2 + ni
                            nsl = bass.ts(n, 512)
                            for blk in range(2):
                                g = 2 * kc + blk
                                silu_evac(y1[g][:, nsl], pts[ni][blk],
                                          b1_t[g], "c1")
                                nc.vector.bn_stats(
                                    out=bst3[g][:, n, :],
                                    in_=y1[g][:, nsl])
                                with nc.allow_low_precision(
                                        reason="bf16 pool partials"):
                                    nc.vector.tensor_reduce(
                                        out=pa[g][:, n * 64:(n + 1) * 64],
                                        in_=y1[g][:, nsl]
                                        .rearrange("p (a w2) -> p a w2", w2=WS),
                                        axis=AX.X, op=ALU.add)
                mv3 = []
                for g in range(4):
                    mv = sm.tile([128, 2], F32, tag="mv3", bufs=4,
                                 name=f"mv3_{b}_{g}")
                    nc.vector.bn_aggr(out=mv, in_=bst3[g])
                    mv3.append(mv)
                sc3 = gn_scale_bias(mv3, g3_t, r3_t, 2, "gn3", ncols=3)

                nc.leave_named_scope(f"conv1_{b}", _s, False)
                _s, _ = nc.enter_named_scope(f"attn_{b}", False)
                # ---- window mean (finish) + radix amax, GN3 folded in ----
                pooled = [sm.tile([128, Hn, Wn], F32, tag="pooled", bufs=2,
                                  name=f"pooled{b}_{i}") for i in range(4)]
                for g in range(4):
                    pav = pa[g].rearrange("p (hn h2 wn) -> p hn wn h2",
                                          hn=Hn, h2=WS)
                    nc.vector.tensor_reduce(out=pooled[g], in_=pav,
                                            axis=AX.X, op=ALU.add)
                pooledT = [sm.tile([64, 128], F32, tag="pooledT", bufs=2,
                                   name=f"pooledT{b}_{i}") for i in range(4)]
                for g in range(4):
                    ptp = psp.tile([64, 128], F32, tag="tp", bufs=1)
                    nc.tensor.transpose(
                        ptp, pooled[g].rearrange("p a b -> p (a b)"), ident)
                    nc.scalar.copy(out=pooledT[g], in_=ptp)
                amT = sm.tile([64, 256], F32, tag="amT", bufs=1)
                for g in range(4):
                    pv = pooledT[g].rearrange("p (a b) -> p a b", b=2)
                    nc.vector.tensor_tensor(
                        out=amT[:, g * 64:(g + 1) * 64],
                        in0=pv[:, :, 0], in1=pv[:, :, 1], op=ALU.max)
                am = [sm.tile([128, 64], F32, tag="am", bufs=2,
                              name=f"am{b}_{i}") for i in range(2)]
                s64 = [sm.tile([128, 1], F32, tag="s64", bufs=2,
                               name=f"s64_{b}_{i}") for i in range(2)]
                for c in range(2):
                    pta = psp.tile([128, 64], F32, tag="tp", bufs=1)
                    nc.tensor.transpose(pta, amT[:, c * 128:(c + 1) * 128],
                                        ident[0:64, 0:64])
                    nc.scalar.copy(out=am[c], in_=pta)
                    # normalize the pooled maxima: am = am*(s3/64) + t3
                    nc.vector.tensor_scalar(
                        out=s64[c], in0=sc3[2 * c][:, 0:1],
                        scalar1=1.0 / (WS * WS), scalar2=None, op0=ALU.mult)
                    nc.vector.tensor_scalar(
                        out=am[c], in0=am[c], scalar1=s64[c],
                        scalar2=sc3[2 * c][:, 1:2], op0=ALU.mult, op1=ALU.add)

                # ---- conv2 (1x1 g=2, 256->64) + silu ----
                p2 = psp.tile([128, 64], F32, tag="tp", bufs=1)
                for g in range(2):
                    nc.tensor.matmul(p2[g * 32:(g + 1) * 32, :], w2_t[g], am[g],
                                     start=True, stop=True)
                a2 = sm.tile([128, 64], F32, tag="a2", bufs=2)
                nc.vector.memset(a2, 0.0)
                silu_evac(a2[0:64, :], p2[0:64, :], b2_t[0:64], "c2")

                # ---- GN4 -> a2n ----
                mv4pad = sm.tile([128, 2], F32, tag="mv4", bufs=2)
                nc.vector.memset(mv4pad, 0.0)
                bst4 = sm.tile([128, 1, 6], F32, tag="bst4", bufs=2)
                nc.vector.bn_stats(out=bst4[0:64], in_=a2[0:64].unsqueeze(1))
                nc.vector.bn_aggr(out=mv4pad[0:64], in_=bst4[0:64])
                sc4 = gn_scale_bias([mv4pad], [g4_t], [r4_t], 8, "gn4")[0]
                a2n = sm.tile([128, 64], F32, tag="a2n", bufs=2)
                nc.vector.memset(a2n, 0.0)
                nc.vector.tensor_scalar(
                    out=a2n[0:64], in0=a2[0:64],
                    scalar1=sc4[0:64, 0:1], scalar2=sc4[0:64, 1:2],
                    op0=ALU.mult, op1=ALU.add)

                # ---- conv3 (1x1 g=2, 64->512), b3 = 0 ----
                a3T = sm.tile([64, 512], F32, tag="a3T", bufs=1)
                for g in range(4):
                    p3 = psp.tile([128, 64], F32, tag="tp", bufs=1)
                    nc.tensor.matmul(p3, w3_t[g], a2n, start=True, stop=True)
                    a3 = sm.tile([128, 64], F32, tag="a3", bufs=2)
                    nc.scalar.copy(out=a3, in_=p3)
                    p3t = psp.tile([64, 128], F32, tag="tp", bufs=1)
                    nc.tensor.transpose(p3t, a3, ident)
                    nc.scalar.copy(out=a3T[:, g * 128:(g + 1) * 128], in_=p3t)

                # ---- softmax over radix == sigmoid of pair difference ----
                a3v = a3T.rearrange("p (a b) -> p a b", b=2)
                dT = sm.tile([64, 256], F32, tag="amT", bufs=1)
                nc.vector.tensor_tensor(out=dT, in0=a3v[:, :, 0],
                                        in1=a3v[:, :, 1], op=ALU.subtract)
                sT = sm.tile([64, 512], F32, tag="sT", bufs=1)
                sTv = sT.rearrange("p (a b) -> p a b", b=2)
                nc.scalar.activation(out=sTv[:, :, 0], in_=dT,
                                     func=AF.Sigmoid, scale=1.0)
                nc.scalar.activation(out=sTv[:, :, 1], in_=dT,
                                     func=AF.Sigmoid, scale=-1.0)
                sint = [sm.tile([128, 64], F32, tag="sint", bufs=4,
                                name=f"sint{b}_{i}") for i in range(4)]
                for g in range(4):
                    pst = psp.tile([128, 64], F32, tag="tp", bufs=1)
                    nc.tensor.transpose(pst, sT[:, g * 128:(g + 1) * 128],
                                        ident[0:64, 0:64])
                    nc.scalar.copy(out=sint[g], in_=pst)

                # fold GN3 scale into the final matmul weights
                wds = [sm.tile([128, 256], BF16, tag="wds", bufs=4,
                               name=f"wds{b}_{i}") for i in range(4)]
                for kc in range(4):
                    nc.vector.tensor_scalar_mul(
                        out=wds[kc], in0=wd_t[kc],
                        scalar1=sc3[kc][:, 0:1])

                nc.leave_named_scope(f"attn_{b}", _s, False)
                _s, _ = nc.enter_named_scope(f"final_{b}", False)
                # ---- gated combine + final channel matmul, pipelined ----
                # z = s3 * ((y1 - mean3) * gate); the s3 lives in wds.
                ot = [big.tile([128, NPIX], F32, tag="ot", bufs=2,
                               name=f"ot{b}_{i}") for i in range(2)]
                bst5 = [sm.tile([128, NT, 6], F32, tag="bst5", bufs=2,
                                name=f"bst5_{b}_{i}") for i in range(2)]
                for nq in range(2):
                    for ni in range(4):
                        n = nq * 4 + ni
                        nsl = bass.ts(n, 512)
                        for g in range(4):
                            grow = sm.tile([128, Wn, WS], F32, tag="grow",
                                           bufs=8, name=f"gr{b}_{n}_{g}",
                                           uniquify=True)
                            gv = sint[g][:, n * Wn:(n + 1) * Wn]
                            nc.gpsimd.tensor_copy(
                                out=grow,
                                in_=gv.unsqueeze(2).broadcast_to(
                                    [128, Wn, WS]))
                            gate = grow.rearrange("p a c -> p (a c)")
                            gate = gate.unsqueeze(1).broadcast_to(
                                [128, WS, Wn * WS])
                            yv = y1[g][:, nsl].rearrange(
                                "p (h2 x) -> p h2 x", h2=WS)
                            nc.vector.scalar_tensor_tensor(
                                out=yv, in0=yv,
                                scalar=sc3[g][:, 2:3], in1=gate,
                                op0=ALU.add, op1=ALU.mult)
                    for m in range(2):
                        ptf = [psp.tile([128, 512], F32, tag="acc", bufs=6,
                                        name=f"pcf_{b}_{nq}_{m}_{i}")
                               for i in range(4)]
                        for ni in range(4):
                            n = nq * 4 + ni
                            for kc in range(4):
                                nc.tensor.matmul(
                                    ptf[ni],
                                    wds[kc][:, m * 128:(m + 1) * 128],
                                    y1[kc][:, bass.ts(n, 512)],
                                    start=(kc == 0), stop=(kc == 3))
                        for ni in range(4):
                            n = nq * 4 + ni
                            nsl = bass.ts(n, 512)
                            nc.scalar.copy(out=ot[m][:, nsl],
                                           in_=ptf[ni])
                            nc.vector.bn_stats(out=bst5[m][:, n, :],
                                               in_=ot[m][:, nsl])

                nc.leave_named_scope(f"final_{b}", _s, False)
                _s, _ = nc.enter_named_scope(f"gn5_{b}", False)
                # ---- GN5 + residual ----
                mv5 = []
                for c in range(2):
                    mv = sm.tile([128, 2], F32, tag="mv5", bufs=2,
                                 name=f"mv5_{b}_{c}")
                    nc.vector.bn_aggr(out=mv, in_=bst5[c])
                    mv5.append(mv)
                sc5 = gn_scale_bias(mv5, gm1_t, rep1_t, 32, "gn5")
                ov = out_d[b].rearrange("c h w -> c (h w)")
                for c in range(2):
                    for q in range(4):
                        qsl = bass.ts(q, NPIX // 4)
                        nc.scalar.activation(out=ot[c][:, qsl],
                                             in_=ot[c][:, qsl],
                                             func=AF.Identity,
                                             bias=sc5[c][:, 1:2],
                                             scale=sc5[c][:, 0:1])
                        eng = nc.gpsimd if (c * 4 + q) % 2 else nc.vector
                        eng.tensor_tensor(out=ot[c][:, qsl],
                                          in0=ot[c][:, qsl],
                                          in1=xr[c][:, qsl], op=ALU.add)
                        nc.sync.dma_start(
                            out=ov[c * 128:(c + 1) * 128, qsl],
                            in_=ot[c][:, qsl])
                if True:
                    nc.leave_named_scope(f"gn5_{b}", _s, False)

    nc.compile()
    return nc


# ---------------------------------------------------------------- entry

_CACHE = {}


def _get_nc(sim_safe=False):
    key = bool(sim_safe)
    if key not in _CACHE:
        _CACHE[key] = build_nc(sim_safe=key)
    return _CACHE[key]


def make_in_maps(inputs):
    hs_full = np.ascontiguousarray(inputs["hidden_state"], dtype=np.float32)
    wd = _host_weights(
        np.asarray(inputs["w0"], np.float32), np.asarray(inputs["b0"], np.float32),
        np.asarray(inputs["w1"], np.float32), np.asarray(inputs["b1"], np.float32),
        np.asarray(inputs["w2"], np.float32), np.asarray(inputs["b2"], np.float32),
        np.asarray(inputs["w3"], np.float32), np.asarray(inputs["b3"], np.float32),
        np.asarray(inputs["weight"], np.float32))
    cm = _host_consts()
    cpack, bpack = _pack_consts(wd, cm)
    shared = {"cpack": cpack, "bpack": bpack}
    in_maps = []
    for i in range(NCORES):
        m = dict(shared)
        m["hs"] = np.ascontiguousarray(hs_full[i * BPC:(i + 1) * BPC])
        m["hsb"] = m["hs"].astype(ml_dtypes.bfloat16)
        in_maps.append(m)
    return in_maps


def kernel(**inputs):
    from concourse import bass_utils
    nc = _get_nc(sim_safe=False)
    in_maps = make_in_maps(inputs)
    res = bass_utils.run_bass_kernel_spmd(nc, in_maps,
                                          core_ids=list(range(NCORES)))
    out = np.concatenate([res.results[i]["out"] for i in range(NCORES)], axis=0)
    return out.astype(np.float32)

